# revision 11
# baseline (speedup 1.0000x reference)
"""Distributed causal multi-head attention (GPT-2 style block) for one TRN2 chip.

Sharding over 8 NeuronCores: core c -> (batch b = c//2, head-group g = c%2).
Each core computes QKV for its batch restricted to its 8 heads (tensor-
parallel column split of W_attn), runs causal attention for those heads,
then the pair of cores sharing a batch AllGathers the per-head attention
output (bf16) and each computes a disjoint column slice of the output
projection. Host assembles the full [4, 2048, 1024] output.

Matmul dtypes: f32r (full-rate fp32, ~1e-4 rel err) for QKV + attention,
bf16 for the output projection. Softmax runs without max-subtraction
(logits are bounded), with the denominator computed by augmenting V with
a ones column so P@[V|1] yields both numerator and denominator.
"""
import numpy as np
import ml_dtypes

B, S, D = 4, 2048, 1024
H, HD, HPC = 16, 64, 8
DL = HPC * HD            # 512 local head features per core
P = 128
CW = 512                 # q-chunk width
NQC = S // CW            # 4
NKT = S // P             # 16
KC = D // P              # 8 contraction chunks of 128
GRP = 3                  # k-tiles per score/exp group (3 PSUM banks)
VW = 65                  # per-head V width incl. ones column

_CACHE: dict = {}


def _build(debug=False):
    from concourse import bacc
    import concourse.mybir as mybir
    from concourse.tile import TileContext

    F32, F32R, BF16 = mybir.dt.float32, mybir.dt.float32r, mybir.dt.bfloat16
    AF = mybir.ActivationFunctionType
    ALU = mybir.AluOpType

    nc = bacc.Bacc(trn_type="TRN2", num_devices=8)
    if debug:
        dbg_qk = nc.declare_dram_parameter("dbg_qk", [P, 8, S], F32R, isOutput=True)
        dbg_vp = nc.declare_dram_parameter("dbg_vp", [P, NKT, HPC * VW], F32R, isOutput=True)
        dbg_rc = nc.declare_dram_parameter("dbg_rc", [NQC * HPC, CW], F32, isOutput=True)
        dbg_at = nc.declare_dram_parameter("dbg_at", [NQC, DL, CW], BF16, isOutput=True)
        dbg_ag = nc.declare_dram_parameter("dbg_ag", [NQC, 2 * DL, CW], BF16, isOutput=True)
        dbg_den = nc.declare_dram_parameter("dbg_den", [NQC * HPC, CW], F32, isOutput=True)
        dbg_pt = nc.declare_dram_parameter("dbg_pt", [P, GRP * CW], F32R, isOutput=True)
    xT = nc.declare_dram_parameter("xT", [D, S], F32R, isOutput=False)
    wqkv = nc.declare_dram_parameter("wqkv", [D, 3 * DL], F32R, isOutput=False)
    bqk = nc.declare_dram_parameter("bqk", [P, 8], F32, isOutput=False)
    bv = nc.declare_dram_parameter("bv", [1, DL], F32, isOutput=False)
    wp = nc.declare_dram_parameter("wp", [D, DL], BF16, isOutput=False)
    bp = nc.declare_dram_parameter("bp", [P, 4], F32, isOutput=False)
    maskc = nc.declare_dram_parameter("maskc", [P, 4 * CW], F32R, isOutput=False)
    ones1 = nc.declare_dram_parameter("ones1", [P, NKT * HPC], F32R, isOutput=False)
    out_ext = nc.declare_dram_parameter("out", [DL, S], F32, isOutput=True)

    cc_in = nc.dram_tensor("cc_in", [NQC, DL, CW], BF16)
    cc_out = nc.dram_tensor("cc_out", [NQC, 2 * DL, CW], BF16)
    RG = [[0, 1], [2, 3], [4, 5], [6, 7]]

    with TileContext(nc) as tc:
        with tc.tile_pool(name="const", bufs=1) as constp, \
             tc.tile_pool(name="qkvt", bufs=1) as qkvtp, \
             tc.tile_pool(name="ps3", bufs=2, space="PSUM") as ps3, \
             tc.tile_pool(name="ps1", bufs=2, space="PSUM") as ps1:

            # ---- constants ----
            bqk_t = constp.tile([P, 8], F32)
            nc.sync.dma_start(out=bqk_t[:], in_=bqk[:])
            bp_t = constp.tile([P, 4], F32)
            nc.sync.dma_start(out=bp_t[:], in_=bp[:])
            maskr = constp.tile([P, 4 * CW], F32R)
            nc.sync.dma_start(out=maskr[:], in_=maskc[:])
            bv_stage = constp.tile([1, DL], F32)
            nc.sync.dma_start(out=bv_stage[:], in_=bv[:])
            bias_bc = constp.tile([P, DL], F32)
            nc.gpsimd.partition_broadcast(bias_bc[:], bv_stage[:])

            # ---- long-lived activations ----
            qk_all = qkvtp.tile([P, 8, S], F32R)          # m 0-3: qT, 4-7: kT
            vpad = qkvtp.tile([P, NKT, HPC * VW], F32R)   # v + ones col per head
            nc.sync.dma_start(
                out=vpad[:].rearrange("p nk (h c) -> p (nk h) c", c=VW)[:, :, HD:VW],
                in_=ones1[:].unsqueeze(2))

            # ================= Phase 1: QKV projection =================
            with tc.tile_pool(name="wq", bufs=1) as wqp, \
                 tc.tile_pool(name="xt", bufs=2) as xtp:
                wq_t = wqp.tile([P, KC, 3 * DL], F32R)
                for kc in range(KC):
                    nc.sync.dma_start(out=wq_t[:, kc, :],
                                      in_=wqkv[kc * P:(kc + 1) * P, :])

                for qc in range(NQC):
                    xtr = xtp.tile([P, KC, CW], F32R, tag="xtr")
                    for kc in range(KC):
                        nc.sync.dma_start(
                            out=xtr[:, kc, :],
                            in_=xT[kc * P:(kc + 1) * P, qc * CW:(qc + 1) * CW])

                    # qT / kT: out[m-chunk, s] = wqkv[:, m].T @ xT[:, s]
                    for ms in ([0, 1, 2], [3, 4, 5], [6, 7]):
                        pt = ps3.tile([P, GRP * CW], F32, tag="ps3")
                        for j, m in enumerate(ms):
                            for kc in range(KC):
                                nc.tensor.matmul(
                                    out=pt[:, j * CW:(j + 1) * CW],
                                    lhsT=wq_t[:, kc, m * P:(m + 1) * P],
                                    rhs=xtr[:, kc, :],
                                    start=(kc == 0), stop=(kc == KC - 1))
                        for j, m in enumerate(ms):
                            nc.vector.tensor_scalar_add(
                                out=qk_all[:, m, qc * CW:(qc + 1) * CW],
                                in0=pt[:, j * CW:(j + 1) * CW],
                                scalar1=bqk_t[:, m:m + 1])

                    # v (natural layout): out[s-tile, vfeat] = xT[:, s].T @ wv
                    for sts in ([0, 1, 2], [3]):
                        pt = ps3.tile([P, GRP * CW], F32, tag="ps3")
                        for j, stl in enumerate(sts):
                            for kc in range(KC):
                                nc.tensor.matmul(
                                    out=pt[:, j * CW:(j + 1) * CW],
                                    lhsT=xtr[:, kc, stl * P:(stl + 1) * P],
                                    rhs=wq_t[:, kc, 2 * DL:3 * DL],
                                    start=(kc == 0), stop=(kc == KC - 1))
                        for j, stl in enumerate(sts):
                            st = qc * 4 + stl
                            nc.vector.tensor_tensor(
                                out=vpad[:, st, :].rearrange(
                                    "p (h c) -> p h c", c=VW)[:, :, 0:HD],
                                in0=pt[:, j * CW:(j + 1) * CW].rearrange(
                                    "p (h c) -> p h c", c=HD),
                                in1=bias_bc[:].rearrange(
                                    "p (h c) -> p h c", c=HD),
                                op=ALU.add)

            if debug:
                for m in range(8):
                    nc.sync.dma_start(out=dbg_qk[:, m, :], in_=qk_all[:, m, :])
                for st in range(NKT):
                    nc.sync.dma_start(out=dbg_vp[:, st, :], in_=vpad[:, st, :])

            # ================= Phase 2: attention + AG + proj =================
            with tc.tile_pool(name="wpp", bufs=1) as wpp, \
                 tc.tile_pool(name="ptp", bufs=2) as ptp, \
                 tc.tile_pool(name="atp", bufs=2) as atp, \
                 tc.tile_pool(name="smallp", bufs=2) as smallp, \
                 tc.tile_pool(name="agp", bufs=2) as agp, \
                 tc.tile_pool(name="otp", bufs=2) as otp:

                wp_t = wpp.tile([P, KC, DL], BF16)
                for kc in range(KC):
                    nc.sync.dma_start(out=wp_t[:, kc, :],
                                      in_=wp[kc * P:(kc + 1) * P, :])

                def emit_attention(qc):
                    at_tiles = [None] * 4
                    for h in range(HPC):
                        half = 64 * (h % 2)
                        qs = qk_all[half:half + 64, h // 2, qc * CW:(qc + 1) * CW]
                        pa = ps1.tile([P, CW], F32, tag="pacc")
                        kmax = 4 * (qc + 1)
                        groups = [list(range(s, min(s + GRP, kmax)))
                                  for s in range(0, kmax, GRP)]
                        pending = None  # (group, ptile)

                        def flush(pending, kmax=kmax, pa=pa):
                            g, ptile = pending
                            for j, kt in enumerate(g):
                                if kt >= 4 * qc:
                                    pat = kt - 4 * qc
                                    nc.vector.tensor_mul(
                                        out=ptile[:, j * CW:(j + 1) * CW],
                                        in0=ptile[:, j * CW:(j + 1) * CW],
                                        in1=maskr[:, pat * CW:(pat + 1) * CW])
                            for j, kt in enumerate(g):
                                nc.tensor.matmul(
                                    out=pa[0:VW, :],
                                    lhsT=vpad[:, kt, h * VW:(h + 1) * VW],
                                    rhs=ptile[:, j * CW:(j + 1) * CW],
                                    start=(kt == 0), stop=(kt == kmax - 1))

                        for g in groups:
                            pt = ps3.tile([P, GRP * CW], F32, tag="ps3")
                            for j, kt in enumerate(g):
                                nc.tensor.matmul(
                                    out=pt[:, j * CW:(j + 1) * CW],
                                    lhsT=qk_all[half:half + 64, 4 + h // 2,
                                                kt * P:(kt + 1) * P],
                                    rhs=qs, start=True, stop=True)
                            if pending is not None:
                                flush(pending)
                            w = len(g) * CW
                            ptile = ptp.tile([P, GRP * CW], F32R, tag="pt")
                            nc.scalar.activation(ptile[:, :w], pt[:, :w],
                                                 AF.Exp, scale=0.125)
                            if debug and qc == 0 and h == 0 and g[0] == 0:
                                nc.sync.dma_start(out=dbg_pt[:, :w],
                                                  in_=ptile[:, :w])
                            pending = (g, ptile)
                        flush(pending)

                        # normalize by the ones-row denominator
                        den = smallp.tile([1, CW], F32, tag="den")
                        nc.vector.tensor_copy(out=den[:], in_=pa[64:65, :])
                        if debug:
                            nc.sync.dma_start(
                                out=dbg_den[qc * HPC + h:qc * HPC + h + 1, :],
                                in_=den[:])
                        rc = smallp.tile([1, CW], F32, tag="recip")
                        nc.vector.reciprocal_approx_fast(out=rc[:], in_=den[:])
                        if debug:
                            nc.sync.dma_start(
                                out=dbg_rc[qc * HPC + h:qc * HPC + h + 1, :],
                                in_=rc[:])
                        bc = smallp.tile([64, CW], F32, tag="bcast")
                        nc.gpsimd.partition_broadcast(bc[:], rc[:])
                        if h % 2 == 0:
                            at_tiles[h // 2] = atp.tile(
                                [P, CW], BF16, tag=f"at{h // 2}",
                                name=f"at_{qc}_{h // 2}")
                        nc.vector.tensor_tensor(
                            out=at_tiles[h // 2][half:half + 64, :],
                            in0=pa[0:64, :], in1=bc[:], op=ALU.mult)

                    for t in range(4):
                        nc.sync.dma_start(out=cc_in[qc, t * P:(t + 1) * P, :],
                                          in_=at_tiles[t][:])
                    nc.gpsimd.collective_compute(
                        "AllGather", ALU.bypass, replica_groups=RG,
                        ins=[cc_in[qc]], outs=[cc_out[qc]])
                    if debug:
                        nc.sync.dma_start(out=dbg_at[qc], in_=cc_in[qc])
                        nc.sync.dma_start(out=dbg_ag[qc], in_=cc_out[qc])

                def emit_proj(qc):
                    agt = agp.tile([P, KC, CW], BF16, tag="ag")
                    for kc in range(KC):
                        nc.sync.dma_start(
                            out=agt[:, kc, :],
                            in_=cc_out[qc, kc * P:(kc + 1) * P, :])
                    for od in range(4):
                        pp = ps1.tile([P, CW], F32, tag="pacc")
                        for kc in range(KC):
                            nc.tensor.matmul(
                                out=pp[:],
                                lhsT=wp_t[:, kc, od * P:(od + 1) * P],
                                rhs=agt[:, kc, :],
                                start=(kc == 0), stop=(kc == KC - 1))
                        ot = otp.tile([P, CW], F32, tag="ot")
                        nc.vector.tensor_scalar_add(out=ot[:], in0=pp[:],
                                                    scalar1=bp_t[:, od:od + 1])
                        nc.sync.dma_start(
                            out=out_ext[od * P:(od + 1) * P,
                                        qc * CW:(qc + 1) * CW],
                            in_=ot[:])

                for qc in range(NQC):
                    emit_attention(qc)
                    if qc >= 1:
                        emit_proj(qc - 1)
                emit_proj(NQC - 1)

    nc.finalize()
    return nc


def _get_nc():
    if "nc" not in _CACHE:
        _CACHE["nc"] = _build()
    return _CACHE["nc"]


def _make_mask():
    j = np.arange(4 * CW) % CW
    pat = np.arange(4 * CW) // CW
    p = np.arange(P)[:, None]
    return (j[None, :] >= pat[None, :] * P + p).astype(np.float32)


def make_in_maps(x, W_attn, b_attn, W_proj, b_proj):
    x = np.asarray(x, np.float32)
    W_attn = np.asarray(W_attn, np.float32)
    b_attn = np.asarray(b_attn, np.float32)
    W_proj = np.asarray(W_proj, np.float32)
    b_proj = np.asarray(b_proj, np.float32)
    mask = _make_mask()
    in_maps = []
    for c in range(8):
        b, g = c // 2, c % 2
        sl = slice(g * DL, (g + 1) * DL)
        wqkv_c = np.concatenate([W_attn[:, g * DL:(g + 1) * DL],
                                 W_attn[:, D + g * DL:D + (g + 1) * DL],
                                 W_attn[:, 2 * D + g * DL:2 * D + (g + 1) * DL]],
                                axis=1)
        bqk_c = np.concatenate([b_attn[g * DL:(g + 1) * DL],
                                b_attn[D + g * DL:D + (g + 1) * DL]])
        in_maps.append({
            "xT": np.ascontiguousarray(x[b].T),
            "wqkv": np.ascontiguousarray(wqkv_c),
            "bqk": np.ascontiguousarray(bqk_c.reshape(8, P).T),
            "bv": b_attn[2 * D + g * DL:2 * D + (g + 1) * DL].reshape(1, DL).copy(),
            "wp": np.ascontiguousarray(W_proj[:, sl]).astype(ml_dtypes.bfloat16),
            "bp": np.ascontiguousarray(b_proj[sl].reshape(4, P).T),
            "maskc": mask,
            "ones1": np.ones((P, NKT * HPC), np.float32),
        })
    return in_maps


def assemble(results):
    out = np.empty((B, S, D), np.float32)
    for c in range(8):
        b, g = c // 2, c % 2
        out[b][:, g * DL:(g + 1) * DL] = results[c]["out"].T
    return out


def kernel(x, W_attn, b_attn, W_proj, b_proj):
    from concourse.bass_utils import run_bass_kernel_spmd
    nc = _get_nc()
    in_maps = make_in_maps(x, W_attn, b_attn, W_proj, b_proj)
    res = run_bass_kernel_spmd(nc, in_maps, core_ids=list(range(8)))
    return assemble(res.results)
